# revision 13
# baseline (speedup 1.0000x reference)
"""LstmCellWithProjection kernel for 8 Trainium2 NeuronCores.

Strategy: 8-way tensor-parallel over the 4*CELL gate dimension.
Each core owns a 512-cell slice of the LSTM cell state (and the matching
2048 rows of W_in / W_state / b_state and 512 columns of W_proj).

  - Input projection x @ W_in.T is a big parallel GEMM done once up front
    (per-core output [2048, B*T], kept resident in SBUF as bf16).
  - The T=128 recurrent steps run with everything feature-major
    ("transposed" layout: feature on the 128 SBUF partitions, batch=32 on
    the free axis).  Per step: gates matmul (W_state shard stationary,
    h streaming), fused activations, cell update + clip, projection
    matmul to a per-core PARTIAL h [512, 32], then an 8-core AllReduce
    produces the full h on every core for the next step.
  - Matmul inputs are bf16 (PSUM accumulation in fp32); the cell state,
    gate pre-activations and h are carried in fp32.

Self-contained: hardcodes shapes from the problem spec.
"""

import sys

if '/opt/trn_rl_repo' not in sys.path:
    sys.path.insert(0, '/opt/trn_rl_repo')

import numpy as np
import ml_dtypes

B, T, D = 32, 128, 512
CELL, HID = 4096, 512
G = 4 * CELL            # 16384 gate rows total
N_CORES = 8
GL = G // N_CORES       # 2048 gate rows per core
CL = CELL // N_CORES    # 512 cells per core
MEM_CLIP = 3.0
PROJ_CLIP = 3.0

_cache = {}
EXCHANGE = "AR"      # "AR" (ncfw AllReduce) or "AG" (AllGather + local sum)
AR_DTYPE = "f16"     # payload dtype for AR
BOUNCE = "scalar"    # engine for collective bounce DMAs


def _build():
    import concourse.bacc as bacc
    import concourse.mybir as mybir
    import concourse.tile as tile

    f32 = mybir.dt.float32
    bf16 = mybir.dt.float16  # compute dtype for PE inputs (fp16: 10-bit mantissa)
    AF = mybir.ActivationFunctionType
    ALU = mybir.AluOpType
    AR_DT = f32 if AR_DTYPE == "f32" else bf16

    nc = bacc.Bacc("TRN2", target_bir_lowering=False, debug=False,
                   num_devices=N_CORES)

    xin_d = nc.dram_tensor("xin", [D, B * T], bf16, kind="ExternalInput")
    wst_d = nc.dram_tensor("wst", [HID, GL], bf16, kind="ExternalInput")
    win_d = nc.dram_tensor("win", [D, GL], bf16, kind="ExternalInput")
    wp_d = nc.dram_tensor("wp", [CL, HID], bf16, kind="ExternalInput")
    bias_d = nc.dram_tensor("bias", [128, 16], f32, kind="ExternalInput")
    outh_d = nc.dram_tensor("out_h", [T, 128, 128], f32, kind="ExternalOutput")
    outc_d = nc.dram_tensor("out_c", [128, 128], f32, kind="ExternalOutput")

    NM = GL // 128   # 16 gate M-tiles per core
    NK = HID // 128  # 4 K-tiles over hid
    NC_ = CL // 128  # 4 cell tiles per core
    NH = HID // 128  # 4 hid tiles
    BT = B * T

    with tile.TileContext(nc) as tc:
        with (
            tc.tile_pool(name="stat", bufs=1) as stat,
            tc.tile_pool(name="state", bufs=1) as state,
            tc.tile_pool(name="gps", bufs=2, space="PSUM") as gps,
            tc.tile_pool(name="hps", bufs=2, space="PSUM") as hpsp,
            tc.tile_pool(name="dram", bufs=4, space="DRAM") as dram,
        ):
            # ---- static weights / inputs ----
            wst_sb = stat.tile([128, NK, GL], bf16)
            wp_sb = stat.tile([128, NC_, HID], bf16)
            bias_sb = stat.tile([128, 16], f32)
            xT_sb = stat.tile([128, NM, BT], bf16)

            for kk in range(NK):
                nc.sync.dma_start(wst_sb[:, kk, :], wst_d[128 * kk:128 * (kk + 1), :])
            for ct in range(NC_):
                nc.sync.dma_start(wp_sb[:, ct, :], wp_d[128 * ct:128 * (ct + 1), :])
            nc.sync.dma_start(bias_sb[:], bias_d[:])

            # ---- phase 1: xT[g, bt] = W_in_k @ x^T + b  (bf16, SBUF resident) ----
            NCH = BT // 512
            with tc.tile_pool(name="gemmio", bufs=1) as gio:
                win_sb = gio.tile([128, NK, GL], bf16)
                xin_sb = gio.tile([128, NK, BT], bf16)
                for kk in range(NK):
                    nc.sync.dma_start(win_sb[:, kk, :], win_d[128 * kk:128 * (kk + 1), :])
                    nc.sync.dma_start(xin_sb[:, kk, :], xin_d[128 * kk:128 * (kk + 1), :])
                for m in range(NM):
                    for nch in range(NCH):
                        pg = gps.tile([128, 512], f32, tag="gemm")
                        for kk in range(NK):
                            nc.tensor.matmul(
                                pg[:],
                                win_sb[:, kk, 128 * m:128 * (m + 1)],
                                xin_sb[:, kk, 512 * nch:512 * (nch + 1)],
                                start=(kk == 0), stop=(kk == NK - 1),
                            )
                        nc.vector.tensor_scalar(
                            xT_sb[:, m, 512 * nch:512 * (nch + 1)], pg[:],
                            bias_sb[:, m:m + 1], None, op0=ALU.add,
                        )

            # ---- phase 2: recurrence ----
            work_cm = tc.tile_pool(name="work", bufs=3)
            work = work_cm.__enter__()
            hT_bf = state.tile([128, NK, B], bf16)   # h (replicated), bf16
            cT = state.tile([128, NC_, B], f32)      # cell state slice
            nc.vector.memset(hT_bf[:], 0.0)
            nc.vector.memset(cT[:], 0.0)

            for t in range(T):
                # gates^T [2048, 32] = W_state_k @ h^T   (+ xT_t + b)
                pg = gps.tile([128, NM, B], f32, tag="rec")
                for m in range(NM):
                    for kk in range(NK):
                        nc.tensor.matmul(
                            pg[:, m, :],
                            wst_sb[:, kk, 128 * m:128 * (m + 1)],
                            hT_bf[:, kk, :],
                            start=(kk == 0), stop=(kk == NK - 1),
                        )
                gsum = work.tile([128, NM, B], f32, tag="gsum")
                for blk in range(4):
                    nc.vector.tensor_tensor(
                        gsum[:, 4 * blk:4 * blk + 4, :],
                        pg[:, 4 * blk:4 * blk + 4, :],
                        xT_sb[:, 4 * blk:4 * blk + 4, B * t:B * (t + 1)],
                        op=mybir.AluOpType.add,
                    )
                gact = work.tile([128, NM, B], f32, tag="gact")
                nc.scalar.activation(gact[:, 0:4, :], gsum[:, 0:4, :], AF.Sigmoid)
                nc.scalar.activation(gact[:, 4:8, :], gsum[:, 4:8, :], AF.Sigmoid)
                nc.scalar.activation(gact[:, 8:12, :], gsum[:, 8:12, :], AF.Tanh)
                nc.scalar.activation(gact[:, 12:16, :], gsum[:, 12:16, :], AF.Sigmoid)

                # c = clip(i*m + f*c)
                im = work.tile([128, NC_, B], f32, tag="im")
                fc = work.tile([128, NC_, B], f32, tag="fc")
                nc.vector.tensor_tensor(im[:], gact[:, 0:4, :], gact[:, 8:12, :], op=ALU.mult)
                nc.vector.tensor_tensor(fc[:], gact[:, 4:8, :], cT[:], op=ALU.mult)
                craw = work.tile([128, NC_, B], f32, tag="craw")
                nc.vector.tensor_tensor(craw[:], im[:], fc[:], op=ALU.add)
                nc.vector.tensor_scalar(cT[:], craw[:], MEM_CLIP, -MEM_CLIP,
                                        op0=ALU.min, op1=ALU.max)

                # pre = o * tanh(c)   (bf16 for the proj matmul)
                tc_t = work.tile([128, NC_, B], f32, tag="tc")
                nc.scalar.activation(tc_t[:], cT[:], AF.Tanh)
                pre = work.tile([128, NC_, B], bf16, tag="pre")
                nc.vector.tensor_tensor(pre[:], gact[:, 12:16, :], tc_t[:], op=ALU.mult)

                # h_partial^T [512, 32] = W_proj_k @ pre^T
                hp = hpsp.tile([128, NH, B], f32, tag="hp")
                for ht in range(NH):
                    for ct in range(NC_):
                        nc.tensor.matmul(
                            hp[:, ht, :],
                            wp_sb[:, ct, 128 * ht:128 * (ht + 1)],
                            pre[:, ct, :],
                            start=(ct == 0), stop=(ct == NC_ - 1),
                        )
                if EXCHANGE == "AR":
                    hs = work.tile([128, NH, B], AR_DT, tag="hs")
                    nc.vector.tensor_copy(hs[:], hp[:])
                    cin = dram.tile([128, NH * B], AR_DT, tag="cin")
                    cout = dram.tile([128, NH * B], AR_DT, tag="cout")
                    bounce_eng = {"gpsimd": nc.gpsimd, "scalar": nc.scalar, "sync": nc.sync}[BOUNCE]
                    bounce_eng.dma_start(cin[:], hs[:])
                    nc.gpsimd.collective_compute(
                        "AllReduce", mybir.AluOpType.add,
                        replica_groups=[list(range(N_CORES))],
                        ins=[cin.opt()], outs=[cout.opt()],
                    )
                    hrs = work.tile([128, NH, B], AR_DT, tag="hrs")
                    bounce_eng.dma_start(hrs[:], cout[:])
                else:
                    hs = work.tile([128, NH, B], bf16, tag="hs")
                    nc.vector.tensor_copy(hs[:], hp[:])
                    cin = dram.tile([128, NH * B], bf16, tag="cin")
                    cout = dram.tile([N_CORES * 128, NH * B], bf16, tag="cout")
                    bounce_eng = {"gpsimd": nc.gpsimd, "scalar": nc.scalar, "sync": nc.sync}[BOUNCE]
                    bounce_eng.dma_start(cin[:], hs[:])
                    nc.gpsimd.collective_compute(
                        "AllGather", mybir.AluOpType.bypass,
                        replica_groups=[list(range(N_CORES))],
                        ins=[cin.opt()], outs=[cout.opt()],
                    )
                    hr = work.tile([128, N_CORES, NH * B], bf16, tag="hr")
                    for s in range(N_CORES):
                        bounce_eng.dma_start(hr[:, s, :], cout[128 * s:128 * (s + 1), :])
                    s01 = work.tile([128, NH * B], f32, tag="s01")
                    s23 = work.tile([128, NH * B], f32, tag="s23")
                    s45 = work.tile([128, NH * B], f32, tag="s45")
                    s67 = work.tile([128, NH * B], f32, tag="s67")
                    nc.vector.tensor_tensor(s01[:], hr[:, 0, :], hr[:, 1, :], op=ALU.add)
                    nc.vector.tensor_tensor(s23[:], hr[:, 2, :], hr[:, 3, :], op=ALU.add)
                    nc.vector.tensor_tensor(s45[:], hr[:, 4, :], hr[:, 5, :], op=ALU.add)
                    nc.vector.tensor_tensor(s67[:], hr[:, 6, :], hr[:, 7, :], op=ALU.add)
                    nc.vector.tensor_tensor(s01[:], s01[:], s23[:], op=ALU.add)
                    nc.vector.tensor_tensor(s45[:], s45[:], s67[:], op=ALU.add)
                    hrs = work.tile([128, NH, B], f32, tag="hrs")
                    nc.vector.tensor_tensor(
                        hrs[:].rearrange("p a b -> p (a b)"), s01[:], s45[:], op=ALU.add)

                # h = clip(h_sum); fp32 copy to DRAM out, fp16 copy for next step
                ho = work.tile([128, NH, B], f32, tag="ho")
                nc.gpsimd.tensor_scalar(ho[:], hrs[:], PROJ_CLIP, -PROJ_CLIP,
                                        op0=ALU.min, op1=ALU.max)
                nc.vector.tensor_scalar(hT_bf[:], hrs[:], PROJ_CLIP, -PROJ_CLIP,
                                        op0=ALU.min, op1=ALU.max)
                nc.sync.dma_start(outh_d[t], ho[:])

            nc.sync.dma_start(outc_d[:], cT[:])
            work_cm.__exit__(None, None, None)

    nc.compile()
    return nc


def _prep_inputs(inputs, W_in, W_state, b_state, W_proj):
    bf = np.float16
    x2d = np.ascontiguousarray(
        np.asarray(inputs, np.float32).transpose(1, 0, 2).reshape(T * B, D))
    xinT = np.ascontiguousarray(x2d.T).astype(bf)          # [D, B*T], bt = t*B+b
    W_in = np.asarray(W_in, np.float32)
    W_state = np.asarray(W_state, np.float32)
    b_state = np.asarray(b_state, np.float32)
    W_proj = np.asarray(W_proj, np.float32)

    in_maps = []
    for k in range(N_CORES):
        idx = np.concatenate(
            [np.arange(g * CELL + k * CL, g * CELL + (k + 1) * CL) for g in range(4)])
        wst = np.ascontiguousarray(W_state[idx].T).astype(bf)   # [HID, GL]
        win = np.ascontiguousarray(W_in[idx].T).astype(bf)      # [D, GL]
        b_k = b_state[idx].astype(np.float32)                   # [GL]
        bias = np.ascontiguousarray(b_k.reshape(16, 128).T)     # [128, 16]
        wp = np.ascontiguousarray(
            W_proj[:, k * CL:(k + 1) * CL].T).astype(bf)        # [CL, HID]
        in_maps.append({"xin": xinT, "wst": wst, "win": win,
                        "wp": wp, "bias": bias})
    return in_maps


def kernel(inputs, W_in, W_state, b_state, W_proj):
    import concourse.bass_utils as bass_utils

    if "nc" not in _cache:
        _cache["nc"] = _build()
    nc = _cache["nc"]

    in_maps = _prep_inputs(inputs, W_in, W_state, b_state, W_proj)
    res = bass_utils.run_bass_kernel_spmd(nc, in_maps,
                                          core_ids=list(range(N_CORES)))

    out_h = res.results[0]["out_h"]          # [T, 128, 128]
    outputs = (out_h.reshape(T, 128, 4, 32)
               .transpose(3, 0, 2, 1)
               .reshape(B, T, HID)
               .astype(np.float32))
    hT = outputs[:, -1, :][None].copy()

    c_full = np.empty((B, CELL), np.float32)
    for k in range(N_CORES):
        oc = res.results[k]["out_c"].reshape(128, 4, 32).transpose(2, 1, 0)
        c_full[:, k * CL:(k + 1) * CL] = oc.reshape(B, CL)
    cT = c_full[None]

    return outputs, (hT, cT)


# revision 14
# speedup vs baseline: 4921.4578x; 4921.4578x over previous
"""LstmCellWithProjection kernel for 8 Trainium2 NeuronCores.

Strategy: 8-way tensor-parallel over the 4*CELL gate dimension.
Each core owns a 512-cell slice of the LSTM cell state (and the matching
2048 rows of W_in / W_state / b_state and 512 columns of W_proj).

  - Input projection x @ W_in.T is a big parallel GEMM done once up front
    (per-core output [2048, B*T], kept resident in SBUF as bf16).
  - The T=128 recurrent steps run with everything feature-major
    ("transposed" layout: feature on the 128 SBUF partitions, batch=32 on
    the free axis).  Per step: gates matmul (W_state shard stationary,
    h streaming), fused activations, cell update + clip, projection
    matmul to a per-core PARTIAL h [512, 32], then an 8-core AllReduce
    produces the full h on every core for the next step.
  - Matmul inputs are bf16 (PSUM accumulation in fp32); the cell state,
    gate pre-activations and h are carried in fp32.

Self-contained: hardcodes shapes from the problem spec.
"""

import sys

if '/opt/trn_rl_repo' not in sys.path:
    sys.path.insert(0, '/opt/trn_rl_repo')

import numpy as np
import ml_dtypes

B, T, D = 32, 128, 512
CELL, HID = 4096, 512
G = 4 * CELL            # 16384 gate rows total
N_CORES = 8
GL = G // N_CORES       # 2048 gate rows per core
CL = CELL // N_CORES    # 512 cells per core
MEM_CLIP = 3.0
PROJ_CLIP = 3.0

_cache = {}
EXCHANGE = "AR"      # "AR" (ncfw AllReduce) or "AG" (AllGather + local sum)
AR_DTYPE = "f16"     # payload dtype for AR
BOUNCE = "scalar"    # engine for collective bounce DMAs
REPEAT = 1           # run the recurrence R times (timing instrumentation only)


def _build():
    import concourse.bacc as bacc
    import concourse.mybir as mybir
    import concourse.tile as tile

    f32 = mybir.dt.float32
    bf16 = mybir.dt.float16  # compute dtype for PE inputs (fp16: 10-bit mantissa)
    AF = mybir.ActivationFunctionType
    ALU = mybir.AluOpType
    AR_DT = f32 if AR_DTYPE == "f32" else bf16

    nc = bacc.Bacc("TRN2", target_bir_lowering=False, debug=False,
                   num_devices=N_CORES)

    xin_d = nc.dram_tensor("xin", [D, B * T], bf16, kind="ExternalInput")
    wst_d = nc.dram_tensor("wst", [HID, GL], bf16, kind="ExternalInput")
    win_d = nc.dram_tensor("win", [D, GL], bf16, kind="ExternalInput")
    wp_d = nc.dram_tensor("wp", [CL, HID], bf16, kind="ExternalInput")
    bias_d = nc.dram_tensor("bias", [128, 16], f32, kind="ExternalInput")
    outh_d = nc.dram_tensor("out_h", [T, 128, 128], f32, kind="ExternalOutput")
    outc_d = nc.dram_tensor("out_c", [128, 128], f32, kind="ExternalOutput")

    NM = GL // 128   # 16 gate M-tiles per core
    NK = HID // 128  # 4 K-tiles over hid
    NC_ = CL // 128  # 4 cell tiles per core
    NH = HID // 128  # 4 hid tiles
    BT = B * T

    with tile.TileContext(nc) as tc:
        with (
            tc.tile_pool(name="stat", bufs=1) as stat,
            tc.tile_pool(name="state", bufs=1) as state,
            tc.tile_pool(name="gps", bufs=2, space="PSUM") as gps,
            tc.tile_pool(name="hps", bufs=2, space="PSUM") as hpsp,
            tc.tile_pool(name="dram", bufs=4, space="DRAM") as dram,
        ):
            # ---- static weights / inputs ----
            wst_sb = stat.tile([128, NK, GL], bf16)
            wp_sb = stat.tile([128, NC_, HID], bf16)
            bias_sb = stat.tile([128, 16], f32)
            xT_sb = stat.tile([128, NM, BT], bf16)

            for kk in range(NK):
                nc.sync.dma_start(wst_sb[:, kk, :], wst_d[128 * kk:128 * (kk + 1), :])
            for ct in range(NC_):
                nc.sync.dma_start(wp_sb[:, ct, :], wp_d[128 * ct:128 * (ct + 1), :])
            nc.sync.dma_start(bias_sb[:], bias_d[:])

            # ---- phase 1: xT[g, bt] = W_in_k @ x^T + b  (bf16, SBUF resident) ----
            NCH = BT // 512
            with tc.tile_pool(name="gemmio", bufs=1) as gio:
                win_sb = gio.tile([128, NK, GL], bf16)
                xin_sb = gio.tile([128, NK, BT], bf16)
                for kk in range(NK):
                    nc.sync.dma_start(win_sb[:, kk, :], win_d[128 * kk:128 * (kk + 1), :])
                    nc.sync.dma_start(xin_sb[:, kk, :], xin_d[128 * kk:128 * (kk + 1), :])
                for m in range(NM):
                    for nch in range(NCH):
                        pg = gps.tile([128, 512], f32, tag="gemm")
                        for kk in range(NK):
                            nc.tensor.matmul(
                                pg[:],
                                win_sb[:, kk, 128 * m:128 * (m + 1)],
                                xin_sb[:, kk, 512 * nch:512 * (nch + 1)],
                                start=(kk == 0), stop=(kk == NK - 1),
                            )
                        nc.vector.tensor_scalar(
                            xT_sb[:, m, 512 * nch:512 * (nch + 1)], pg[:],
                            bias_sb[:, m:m + 1], None, op0=ALU.add,
                        )

            # ---- phase 2: recurrence ----
            work_cm = tc.tile_pool(name="work", bufs=3)
            work = work_cm.__enter__()
            hT_bf = state.tile([128, NK, B], bf16)   # h (replicated), bf16
            cT = state.tile([128, NC_, B], f32)      # cell state slice

            for _rep in range(REPEAT):
              nc.vector.memset(hT_bf[:], 0.0)
              nc.vector.memset(cT[:], 0.0)

              for t in range(T):
                # gates^T [2048, 32] = W_state_k @ h^T   (+ xT_t + b)
                pg = gps.tile([128, NM, B], f32, tag="rec")
                for m in range(NM):
                    for kk in range(NK):
                        nc.tensor.matmul(
                            pg[:, m, :],
                            wst_sb[:, kk, 128 * m:128 * (m + 1)],
                            hT_bf[:, kk, :],
                            start=(kk == 0), stop=(kk == NK - 1),
                        )
                gsum = work.tile([128, NM, B], f32, tag="gsum")
                for blk in range(4):
                    nc.vector.tensor_tensor(
                        gsum[:, 4 * blk:4 * blk + 4, :],
                        pg[:, 4 * blk:4 * blk + 4, :],
                        xT_sb[:, 4 * blk:4 * blk + 4, B * t:B * (t + 1)],
                        op=mybir.AluOpType.add,
                    )
                gact = work.tile([128, NM, B], f32, tag="gact")
                nc.scalar.activation(gact[:, 0:4, :], gsum[:, 0:4, :], AF.Sigmoid)
                nc.scalar.activation(gact[:, 4:8, :], gsum[:, 4:8, :], AF.Sigmoid)
                nc.scalar.activation(gact[:, 8:12, :], gsum[:, 8:12, :], AF.Tanh)
                nc.scalar.activation(gact[:, 12:16, :], gsum[:, 12:16, :], AF.Sigmoid)

                # c = clip(i*m + f*c)
                im = work.tile([128, NC_, B], f32, tag="im")
                fc = work.tile([128, NC_, B], f32, tag="fc")
                nc.vector.tensor_tensor(im[:], gact[:, 0:4, :], gact[:, 8:12, :], op=ALU.mult)
                nc.vector.tensor_tensor(fc[:], gact[:, 4:8, :], cT[:], op=ALU.mult)
                craw = work.tile([128, NC_, B], f32, tag="craw")
                nc.vector.tensor_tensor(craw[:], im[:], fc[:], op=ALU.add)
                nc.vector.tensor_scalar(cT[:], craw[:], MEM_CLIP, -MEM_CLIP,
                                        op0=ALU.min, op1=ALU.max)

                # pre = o * tanh(c)   (bf16 for the proj matmul)
                tc_t = work.tile([128, NC_, B], f32, tag="tc")
                nc.scalar.activation(tc_t[:], cT[:], AF.Tanh)
                pre = work.tile([128, NC_, B], bf16, tag="pre")
                nc.vector.tensor_tensor(pre[:], gact[:, 12:16, :], tc_t[:], op=ALU.mult)

                # h_partial^T [512, 32] = W_proj_k @ pre^T
                hp = hpsp.tile([128, NH, B], f32, tag="hp")
                for ht in range(NH):
                    for ct in range(NC_):
                        nc.tensor.matmul(
                            hp[:, ht, :],
                            wp_sb[:, ct, 128 * ht:128 * (ht + 1)],
                            pre[:, ct, :],
                            start=(ct == 0), stop=(ct == NC_ - 1),
                        )
                if EXCHANGE == "AR":
                    hs = work.tile([128, NH, B], AR_DT, tag="hs")
                    nc.vector.tensor_copy(hs[:], hp[:])
                    cin = dram.tile([128, NH * B], AR_DT, tag="cin")
                    cout = dram.tile([128, NH * B], AR_DT, tag="cout")
                    bounce_eng = {"gpsimd": nc.gpsimd, "scalar": nc.scalar, "sync": nc.sync}[BOUNCE]
                    bounce_eng.dma_start(cin[:], hs[:])
                    nc.gpsimd.collective_compute(
                        "AllReduce", mybir.AluOpType.add,
                        replica_groups=[list(range(N_CORES))],
                        ins=[cin.opt()], outs=[cout.opt()],
                    )
                    hrs = work.tile([128, NH, B], AR_DT, tag="hrs")
                    bounce_eng.dma_start(hrs[:], cout[:])
                else:
                    hs = work.tile([128, NH, B], bf16, tag="hs")
                    nc.vector.tensor_copy(hs[:], hp[:])
                    cin = dram.tile([128, NH * B], bf16, tag="cin")
                    cout = dram.tile([N_CORES * 128, NH * B], bf16, tag="cout")
                    bounce_eng = {"gpsimd": nc.gpsimd, "scalar": nc.scalar, "sync": nc.sync}[BOUNCE]
                    bounce_eng.dma_start(cin[:], hs[:])
                    nc.gpsimd.collective_compute(
                        "AllGather", mybir.AluOpType.bypass,
                        replica_groups=[list(range(N_CORES))],
                        ins=[cin.opt()], outs=[cout.opt()],
                    )
                    hr = work.tile([128, N_CORES, NH * B], bf16, tag="hr")
                    for s in range(N_CORES):
                        bounce_eng.dma_start(hr[:, s, :], cout[128 * s:128 * (s + 1), :])
                    s01 = work.tile([128, NH * B], f32, tag="s01")
                    s23 = work.tile([128, NH * B], f32, tag="s23")
                    s45 = work.tile([128, NH * B], f32, tag="s45")
                    s67 = work.tile([128, NH * B], f32, tag="s67")
                    nc.vector.tensor_tensor(s01[:], hr[:, 0, :], hr[:, 1, :], op=ALU.add)
                    nc.vector.tensor_tensor(s23[:], hr[:, 2, :], hr[:, 3, :], op=ALU.add)
                    nc.vector.tensor_tensor(s45[:], hr[:, 4, :], hr[:, 5, :], op=ALU.add)
                    nc.vector.tensor_tensor(s67[:], hr[:, 6, :], hr[:, 7, :], op=ALU.add)
                    nc.vector.tensor_tensor(s01[:], s01[:], s23[:], op=ALU.add)
                    nc.vector.tensor_tensor(s45[:], s45[:], s67[:], op=ALU.add)
                    hrs = work.tile([128, NH, B], f32, tag="hrs")
                    nc.vector.tensor_tensor(
                        hrs[:].rearrange("p a b -> p (a b)"), s01[:], s45[:], op=ALU.add)

                # h = clip(h_sum); fp32 copy to DRAM out, fp16 copy for next step
                ho = work.tile([128, NH, B], f32, tag="ho")
                nc.gpsimd.tensor_scalar(ho[:], hrs[:], PROJ_CLIP, -PROJ_CLIP,
                                        op0=ALU.min, op1=ALU.max)
                nc.vector.tensor_scalar(hT_bf[:], hrs[:], PROJ_CLIP, -PROJ_CLIP,
                                        op0=ALU.min, op1=ALU.max)
                nc.sync.dma_start(outh_d[t], ho[:])

            nc.sync.dma_start(outc_d[:], cT[:])
            work_cm.__exit__(None, None, None)

    nc.compile()
    return nc


def _prep_inputs(inputs, W_in, W_state, b_state, W_proj):
    bf = np.float16
    x2d = np.ascontiguousarray(
        np.asarray(inputs, np.float32).transpose(1, 0, 2).reshape(T * B, D))
    xinT = np.ascontiguousarray(x2d.T).astype(bf)          # [D, B*T], bt = t*B+b
    W_in = np.asarray(W_in, np.float32)
    W_state = np.asarray(W_state, np.float32)
    b_state = np.asarray(b_state, np.float32)
    W_proj = np.asarray(W_proj, np.float32)

    in_maps = []
    for k in range(N_CORES):
        idx = np.concatenate(
            [np.arange(g * CELL + k * CL, g * CELL + (k + 1) * CL) for g in range(4)])
        wst = np.ascontiguousarray(W_state[idx].T).astype(bf)   # [HID, GL]
        win = np.ascontiguousarray(W_in[idx].T).astype(bf)      # [D, GL]
        b_k = b_state[idx].astype(np.float32)                   # [GL]
        bias = np.ascontiguousarray(b_k.reshape(16, 128).T)     # [128, 16]
        wp = np.ascontiguousarray(
            W_proj[:, k * CL:(k + 1) * CL].T).astype(bf)        # [CL, HID]
        in_maps.append({"xin": xinT, "wst": wst, "win": win,
                        "wp": wp, "bias": bias})
    return in_maps


def kernel(inputs, W_in, W_state, b_state, W_proj):
    import concourse.bass_utils as bass_utils

    if "nc" not in _cache:
        _cache["nc"] = _build()
    nc = _cache["nc"]

    in_maps = _prep_inputs(inputs, W_in, W_state, b_state, W_proj)
    res = bass_utils.run_bass_kernel_spmd(nc, in_maps,
                                          core_ids=list(range(N_CORES)))

    out_h = res.results[0]["out_h"]          # [T, 128, 128]
    outputs = (out_h.reshape(T, 128, 4, 32)
               .transpose(3, 0, 2, 1)
               .reshape(B, T, HID)
               .astype(np.float32))
    hT = outputs[:, -1, :][None].copy()

    c_full = np.empty((B, CELL), np.float32)
    for k in range(N_CORES):
        oc = res.results[k]["out_c"].reshape(128, 4, 32).transpose(2, 1, 0)
        c_full[:, k * CL:(k + 1) * CL] = oc.reshape(B, CL)
    cT = c_full[None]

    return outputs, (hT, cT)
